# revision 31
# baseline (speedup 1.0000x reference)
"""Binary-weight 3x3 conv (sign(weight) then conv2d, pad=1) on 8 TRN2 cores.

v2: F(2,3) Winograd along H, direct 3-tap conv along W, fp16 datapath.

Data-parallel over batch: 32 images -> 4 per core; weights replicated.
Per core the conv is decomposed as:
  - input transform along H (DVE): V[i][ci, ty, w] = BT(F2,3) combos of
    x rows 2ty-1..2ty+2 (4 plain tensor_tensor adds per (img, ih-half),
    all +-1 coefficients, fp16 2x perf mode).
  - main matmul (PE): M[i][co, ty, w] = sum_{ci, kw} U[i][co, ci, kw] *
    V[i][ci, ty, w+kw-1], with U = G(F2,3) @ sign(w) along kh (entries in
    {0, +-0.5, +-1, +-1.5}: exact in fp16, packed on host). MAC count is
    2/3 of direct conv: per (img, oh, 7-ty chunk) 24 matmuls of
    [128x128]@[128x392] instead of direct conv's 36.
  - drains (ACT): plain PSUM->SBUF fp16 copies.
  - output transform (DVE): Y[2ty] = m0+m1+m2, Y[2ty+1] = m1-m2-m3
    (4 tensor_tensor ops per (img, oh) over all 28 ty), written
    row-interleaved to fp16 staging; DMA'd out fp16, widened on host.

x is uploaded as fp16 (host cast, like the host-packed weights).
Numerics (numpy bit-accurate sim): rel err 5.4e-4 vs fp32 reference.

Layouts per (img, ih):
  xpad [128, 3304]: H-padded rows r in [-1,56] at offset (r+1)*56 (rows -1
    and 56 zeroed once; 56 trailing slack elems for AP slicing).
  V [128, 4*28*60]: plane i, tile row ty at (i*28+ty)*60; index 2+w holds
    col w in [0,56); indices 1 and 58 are zero W-padding (memset once;
    tap kw reads cols kw+1 .. kw+57 of each row).
"""

import numpy as np

import concourse.bacc as bacc
import concourse.mybir as mybir
import concourse.tile as tile
from concourse.bass_utils import run_bass_kernel_spmd

MODE = "f23h"  # kept for test.py compat

N_CORES = 8
B = 32
BPC = B // N_CORES  # images per core
C = 256
H = W = 56
HW = H * W  # 3136
TY = 28          # H tiles (2 output rows each)
NPL = 4          # transformed planes
VROW = 60        # V row width (2 pad + 56 + 2 pad)
XIN = 58 * W              # 3248: x uploaded host-prepadded (rows -1..56)
XSZ = XIN + W             # 3304: + slack for AP slicing (never read)
VSZ = NPL * TY * VROW     # 6720
OSZ = HW + W              # 3192: osb + slack for strided row writes
TYC = 7          # ty per chunk
NCH = TY // TYC  # 4 chunks per (img, oh)
NF = TYC * W     # 392 matmul free size
MW = NPL * 3 * 2 * 128    # weight cols per oh half
NWARM = 68

_CACHE = {}


def _build_module():
    fp16 = mybir.dt.float16
    nc = bacc.Bacc("TRN2", target_bir_lowering=False, debug=False, num_devices=N_CORES)
    x = nc.declare_dram_parameter("x", [BPC, C, XIN], fp16, isOutput=False)
    wt = nc.declare_dram_parameter("wt", [2, 128, MW], fp16, isOutput=False)
    out = nc.declare_dram_parameter("out", [BPC, C, HW], fp16, isOutput=True)

    with tile.TileContext(nc) as tc:
        with (
            tc.tile_pool(name="xpads", bufs=4) as xpool,
            tc.tile_pool(name="vts", bufs=4) as vpool,
            tc.tile_pool(name="wts", bufs=2) as wpool,
            tc.tile_pool(name="msbs", bufs=3) as mpool,
            tc.tile_pool(name="osbs", bufs=3) as opool,
            tc.tile_pool(name="oscr", bufs=4) as spool,
            tc.tile_pool(name="psum", bufs=8, space="PSUM") as ppool,
        ):
            xpads = [xpool.tile([128, XSZ], fp16, tag="xpad", name=f"xpad_{j}")
                     for j in range(4)]
            vts = [vpool.tile([128, VSZ], fp16, tag="vt", name=f"vt_{j}")
                   for j in range(4)]
            wts = [wpool.tile([128, MW], fp16, tag="wt", name=f"wt_{oh}")
                   for oh in range(2)]
            msbs = [mpool.tile([128, NPL * TY * W], fp16, tag="msb", name=f"msb_{j}")
                    for j in range(3)]
            osbs = [opool.tile([128, OSZ], fp16, tag="osb", name=f"osb_{j}")
                    for j in range(3)]
            oscr = [spool.tile([128, TY * W], fp16, tag="oscr", name=f"oscr_{j}")
                    for j in range(4)]
            psts = [ppool.tile([128, NF], mybir.dt.float32, tag="ps", name=f"ps_{j}")
                    for j in range(8)]
            warm_sb = wpool.tile([128, 128], fp16, tag="warm")

            # one-time zeroing: V pad columns (strided 2-col, cheap) and xpad
            # H-pad rows. Only image-0's buffers (0,1) upfront; buffers 2,3
            # are zeroed after image-0's transform is queued (deferred_zero).
            nc.vector.memset(warm_sb[:], 0.0)

            def zero_pads(j, eng=None):
                eng = eng or nc.vector
                vrr = vts[j][:].rearrange("p (t w) -> p t w", w=VROW)
                eng.memset(vrr[:, :, 0:2], 0.0)
                eng.memset(vrr[:, :, 58:60], 0.0)
                # xpad slack tail: written once so AP bounding slices are clean
                eng.memset(xpads[j][:, XIN:XSZ], 0.0)

            zero_pads(0)
            zero_pads(1)

            def load_x(n, ih):
                t = xpads[(n * 2 + ih) % 4]
                eng = nc.scalar if ih == 0 else nc.sync
                eng.dma_start(t[:, 0:XIN], x.ap()[n, ih * 128 : (ih + 1) * 128, :])

            # critical path: image 0 rows 0..29 (covers V chunks 0-1) first
            # on both rings, then weights, then the rest of image 0; image 1
            # is loaded later (from the first chunk-0 block) so its DMA
            # semaphore increments can't inflate image-0 waits
            def load_x0_rows(ih, pr0, pr1):
                # pr = PADDED row indices (0..58); pad rows come from the DMA
                t = xpads[ih % 4]
                eng = nc.scalar if ih == 0 else nc.sync
                eng.dma_start(t[:, pr0 * W : pr1 * W],
                              x.ap()[0, ih * 128 : (ih + 1) * 128, pr0 * W : pr1 * W])

            load_x0_rows(0, 0, 18)
            load_x0_rows(1, 0, 18)
            load_x0_rows(0, 18, 31)
            load_x0_rows(1, 18, 31)
            nc.scalar.dma_start(wts[0][:], wt.ap()[0])
            nc.sync.dma_start(wts[1][:], wt.ap()[1])
            load_x0_rows(0, 31, 58)
            load_x0_rows(1, 31, 58)

            # PE warmup while DMAs land (HAM clock gate 1.2->2.4 GHz)
            warm_ps = psts[7]
            for _ in range(NWARM):
                nc.tensor.matmul(warm_ps[:, 0:128], lhsT=warm_sb[:], rhs=warm_sb[:],
                                 start=True, stop=True)

            def drows(xp, h, ty0, nty):
                # x rows (2*ty + h - 1) for ty in [ty0, ty0+nty): AP [nty, 56]
                off = (2 * ty0 + h) * W
                return xp[:, off : off + nty * 2 * W].rearrange(
                    "p (t w) -> p t w", w=2 * W)[:, :, 0:W]

            def vrows(vt, i, ty0, nty):
                off = (i * TY + ty0) * VROW
                return vt[:, off : off + nty * VROW].rearrange(
                    "p (t w) -> p t w", w=VROW)[:, :, 2 : 2 + W]

            def vop(n, ih, i, ty0, nty):
                xp = xpads[(n * 2 + ih) % 4]
                vt = vts[(n * 2 + ih) % 4]
                d = [drows(xp, h, ty0, nty) for h in range(4)]
                dst = vrows(vt, i, ty0, nty)
                if i == 0:
                    nc.vector.tensor_sub(dst, d[0], d[2])
                elif i == 1:
                    nc.vector.tensor_add(dst, d[1], d[2])
                elif i == 2:
                    nc.vector.tensor_sub(dst, d[2], d[1])
                else:
                    nc.vector.tensor_sub(dst, d[1], d[3])

            def emit_v(n, ih, ty0, nty):
                for i in range(NPL):
                    vop(n, ih, i, ty0, nty)

            def mm_rhs(n, ihf, i, ch, kw):
                vt = vts[(n * 2 + ihf) % 4]
                off = (i * TY + ch * TYC) * VROW
                return vt[:, off : off + TYC * VROW].rearrange(
                    "p (t w) -> p t w", w=VROW)[:, :, kw + 1 : kw + 1 + W]

            # startup: V for image 0 chunk 0 only — the rest is emitted
            # interleaved with chunk matmuls so Tile's engine-semaphore
            # waits stay tight (emitting it all upfront made the first
            # matmuls wait on the whole image-0 transform)
            for i in range(NPL):
                vop(0, 0, i, 0, TYC)
                vop(0, 1, i, 0, TYC)

            def yrows(osb, s, ty0, nty):
                off = ty0 * 2 * W + s * W
                return osb[:, off : off + nty * 2 * W].rearrange(
                    "p (t w) -> p t w", w=2 * W)[:, :, 0:W]

            def emit_combos(n, oh, msb, osb, ty0, nty, sidx):
                # Y[2ty] = m0+m1+m2, Y[2ty+1] = m1-m2-m3 over ty-range
                # (msb chunk-major layout: ty-range == chunk-range * NF)
                m = [msb[:, i * TY * W + ty0 * W : i * TY * W + (ty0 + nty) * W]
                     for i in range(NPL)]
                A = oscr[sidx * 2][:, 0 : nty * W]
                Bs = oscr[sidx * 2 + 1][:, 0 : nty * W]
                nc.vector.tensor_add(A, m[0], m[1])
                nc.vector.tensor_add(yrows(osb, 0, ty0, nty), A, m[2])
                nc.vector.tensor_sub(Bs, m[1], m[2])
                nc.vector.tensor_sub(yrows(osb, 1, ty0, nty), Bs, m[3])

            pp = 0
            for n in range(BPC):
                for oh in range(2):
                    last = (n == BPC - 1 and oh == 1)
                    first = (n == 0 and oh == 0)
                    msb = msbs[(n * 2 + oh) % 3]
                    osb = osbs[(n * 2 + oh) % 3]
                    if not (last or first):
                        for i in range(NPL):
                            # weight-reuse order: same lhsT serves all 4 chunks
                            k = 0
                            for kw in range(3):
                                for ihf in range(2):
                                    c0 = ((i * 3 + kw) * 2 + ihf) * 128
                                    for ch in range(NCH):
                                        nc.tensor.matmul(
                                            psts[(pp + ch) % 8][:],
                                            lhsT=wts[oh][:, c0 : c0 + 128],
                                            rhs=mm_rhs(n, ihf, i, ch, kw),
                                            start=(k == 0), stop=(k == 5))
                                    k += 1
                            for ch in range(NCH):
                                nc.scalar.copy(
                                    msb[:, (i * NCH + ch) * NF :
                                         (i * NCH + ch + 1) * NF],
                                    psts[(pp + ch) % 8][:])
                            pp += NCH
                        emit_combos(n, oh, msb, osb, 0, TY, (n * 2 + oh) % 2)
                        nc.sync.dma_start(
                            out.ap()[n, oh * 128 : (oh + 1) * 128, :],
                            osb[:, 0:HW])
                    else:
                        # first/last (img, oh): chunk-outer so the first
                        # chunk can start before the whole image's V exists
                        # (first) and drains/combos/DMA overlap the matmul
                        # stream, keeping the tail short (last)
                        for ch in range(NCH):
                            tail_ch = last and ch == NCH - 1
                            for i in range(NPL):
                                P = psts[pp % 8]
                                pp += 1
                                k = 0
                                for kw in range(3):
                                    for ihf in range(2):
                                        c0 = ((i * 3 + kw) * 2 + ihf) * 128
                                        nc.tensor.matmul(
                                            P[:], lhsT=wts[oh][:, c0 : c0 + 128],
                                            rhs=mm_rhs(n, ihf, i, ch, kw),
                                            start=(k == 0), stop=(k == 5))
                                        k += 1
                                ms = msb[:, (i * NCH + ch) * NF :
                                         (i * NCH + ch + 1) * NF]
                                if tail_ch and i == NPL - 1:
                                    # last plane drains on DVE: shorter
                                    # PSUM->combo chain after the final MM
                                    nc.vector.tensor_copy(ms, P[:])
                                else:
                                    nc.scalar.copy(ms, P[:])
                            if first:
                                # keep the DVE free for the next chunk's V;
                                # combos for the whole image run later
                                if ch == 0:
                                    zero_pads(2, nc.gpsimd)
                                    zero_pads(3, nc.gpsimd)
                                    load_x(1, 0)
                                    load_x(1, 1)
                                if ch + 1 < NCH:
                                    emit_v(0, 0, (ch + 1) * TYC, TYC)
                                    emit_v(0, 1, (ch + 1) * TYC, TYC)
                                continue
                            ty0 = ch * TYC
                            m = [msb[:, i * TY * W + ty0 * W :
                                     i * TY * W + (ty0 + TYC) * W]
                                 for i in range(NPL)]
                            A = oscr[(ch % 2) * 2][:, 0 : TYC * W]
                            Bs = oscr[(ch % 2) * 2 + 1][:, 0 : TYC * W]
                            nc.vector.tensor_add(A, m[0], m[1])
                            nc.vector.tensor_sub(Bs, m[1], m[2])
                            nc.vector.tensor_add(yrows(osb, 0, ty0, TYC), A, m[2])
                            nc.vector.tensor_sub(yrows(osb, 1, ty0, TYC), Bs, m[3])
                            if tail_ch:
                                nc.sync.dma_start(
                                    out.ap()[n, oh * 128 : (oh + 1) * 128,
                                             ch * TYC * 2 * W : HW],
                                    osb[:, ch * TYC * 2 * W : HW])
                            elif last and ch == 2:
                                nc.sync.dma_start(
                                    out.ap()[n, oh * 128 : (oh + 1) * 128,
                                             2 * TYC * 2 * W : 3 * TYC * 2 * W],
                                    osb[:, 2 * TYC * 2 * W : 3 * TYC * 2 * W])
                            elif ch % 2 == 1:
                                nc.sync.dma_start(
                                    out.ap()[n, oh * 128 : (oh + 1) * 128,
                                             (ch - 1) * TYC * 2 * W :
                                             (ch + 1) * TYC * 2 * W],
                                    osb[:, (ch - 1) * TYC * 2 * W :
                                         (ch + 1) * TYC * 2 * W])
                        if first:
                            emit_combos(n, oh, msb, osb, 0, TY, 0)
                            nc.sync.dma_start(
                                out.ap()[n, oh * 128 : (oh + 1) * 128, :],
                                osb[:, 0:HW])
                    # prefetch next image's V (and x two images ahead)
                    if oh == 0 and n + 1 < BPC:
                        emit_v(n + 1, 0, 0, TY)
                        emit_v(n + 1, 1, 0, TY)
                        if n + 2 < BPC:
                            load_x(n + 2, 0)
                            load_x(n + 2, 1)

    nc.compile()
    return nc


def _pack_weights(weight: np.ndarray) -> np.ndarray:
    bw = np.sign(weight.astype(np.float32))  # [co 256, ci 256, kh 3, kw 3]
    G23 = np.array([[1, 0, 0], [0.5, 0.5, 0.5], [0.5, -0.5, 0.5], [0, 0, 1]],
                   dtype=np.float32)
    U = np.einsum("ik,ockw->iocw", G23, bw)  # [4, co, ci, kw]
    U = U.reshape(NPL, 2, 128, 2, 128, 3)    # [i, ohh, co, ihh, ci, kw]
    U = U.transpose(1, 4, 0, 5, 3, 2)        # [oh, ci, i, kw, ihf, co]
    return np.ascontiguousarray(U.reshape(2, 128, MW)).astype(np.float16)


def _get_nc():
    if "nc" not in _CACHE:
        _CACHE["nc"] = _build_module()
    return _CACHE["nc"]


def _run(x: np.ndarray, weight: np.ndarray, **spmd_kwargs):
    nc = _get_nc()
    wtp = _pack_weights(weight)
    xh = np.zeros((B, C, 58, W), np.float16)
    xh[:, :, 1:57, :] = x.astype(np.float16).reshape(B, C, H, W)
    xh = np.ascontiguousarray(xh.reshape(B, C, 58 * W))
    in_maps = [
        {"x": xh[i * BPC : (i + 1) * BPC], "wt": wtp} for i in range(N_CORES)
    ]
    res = run_bass_kernel_spmd(nc, in_maps, list(range(N_CORES)), **spmd_kwargs)
    out = np.concatenate([r["out"] for r in res.results], axis=0)
    out = out.astype(np.float32).reshape(B, C, H, W)
    return out, res


def kernel(x: np.ndarray, weight: np.ndarray) -> np.ndarray:
    out, _ = _run(x, weight)
    return out


# revision 32
# speedup vs baseline: 1.0080x; 1.0080x over previous
"""Binary-weight 3x3 conv (sign(weight) then conv2d, pad=1) on 8 TRN2 cores.

v2: F(2,3) Winograd along H, direct 3-tap conv along W, fp16 datapath.

Data-parallel over batch: 32 images -> 4 per core; weights replicated.
Per core the conv is decomposed as:
  - input transform along H (DVE): V[i][ci, ty, w] = BT(F2,3) combos of
    x rows 2ty-1..2ty+2 (4 plain tensor_tensor adds per (img, ih-half),
    all +-1 coefficients, fp16 2x perf mode).
  - main matmul (PE): M[i][co, ty, w] = sum_{ci, kw} U[i][co, ci, kw] *
    V[i][ci, ty, w+kw-1], with U = G(F2,3) @ sign(w) along kh (entries in
    {0, +-0.5, +-1, +-1.5}: exact in fp16, packed on host). MAC count is
    2/3 of direct conv: per (img, oh, 7-ty chunk) 24 matmuls of
    [128x128]@[128x392] instead of direct conv's 36.
  - drains (ACT): plain PSUM->SBUF fp16 copies.
  - output transform (DVE): Y[2ty] = m0+m1+m2, Y[2ty+1] = m1-m2-m3
    (4 tensor_tensor ops per (img, oh) over all 28 ty), written
    row-interleaved to fp16 staging; DMA'd out fp16, widened on host.

x is uploaded as fp16 (host cast, like the host-packed weights).
Numerics (numpy bit-accurate sim): rel err 5.4e-4 vs fp32 reference.

Layouts per (img, ih):
  xpad [128, 3304]: H-padded rows r in [-1,56] at offset (r+1)*56 (rows -1
    and 56 zeroed once; 56 trailing slack elems for AP slicing).
  V [128, 4*28*60]: plane i, tile row ty at (i*28+ty)*60; index 2+w holds
    col w in [0,56); indices 1 and 58 are zero W-padding (memset once;
    tap kw reads cols kw+1 .. kw+57 of each row).
"""

import numpy as np

import concourse.bacc as bacc
import concourse.mybir as mybir
import concourse.tile as tile
from concourse.bass_utils import run_bass_kernel_spmd

MODE = "f23h"  # kept for test.py compat

N_CORES = 8
B = 32
BPC = B // N_CORES  # images per core
C = 256
H = W = 56
HW = H * W  # 3136
TY = 28          # H tiles (2 output rows each)
NPL = 4          # transformed planes
VROW = 60        # V row width (2 pad + 56 + 2 pad)
XIN = 58 * W              # 3248: x uploaded host-prepadded (rows -1..56)
XSZ = XIN + W             # 3304: + slack for AP slicing (never read)
VSZ = NPL * TY * VROW     # 6720
OSZ = HW + W              # 3192: osb + slack for strided row writes
TYC = 7          # ty per chunk
NCH = TY // TYC  # 4 chunks per (img, oh)
NF = TYC * W     # 392 matmul free size
MW = NPL * 3 * 2 * 128    # weight cols per oh half
NWARM = 68

_CACHE = {}


def _build_module():
    fp16 = mybir.dt.float16
    nc = bacc.Bacc("TRN2", target_bir_lowering=False, debug=False, num_devices=N_CORES)
    x = nc.declare_dram_parameter("x", [BPC, C, XIN], fp16, isOutput=False)
    wt = nc.declare_dram_parameter("wt", [2, 128, MW], fp16, isOutput=False)
    out = nc.declare_dram_parameter("out", [BPC, C, HW], fp16, isOutput=True)

    with tile.TileContext(nc) as tc:
        with (
            tc.tile_pool(name="xpads", bufs=4) as xpool,
            tc.tile_pool(name="vts", bufs=4) as vpool,
            tc.tile_pool(name="wts", bufs=2) as wpool,
            tc.tile_pool(name="msbs", bufs=3) as mpool,
            tc.tile_pool(name="osbs", bufs=3) as opool,
            tc.tile_pool(name="oscr", bufs=4) as spool,
            tc.tile_pool(name="psum", bufs=8, space="PSUM") as ppool,
        ):
            xpads = [xpool.tile([128, XSZ], fp16, tag="xpad", name=f"xpad_{j}")
                     for j in range(4)]
            vts = [vpool.tile([128, VSZ], fp16, tag="vt", name=f"vt_{j}")
                   for j in range(4)]
            wts = [wpool.tile([128, MW], fp16, tag="wt", name=f"wt_{oh}")
                   for oh in range(2)]
            msbs = [mpool.tile([128, NPL * TY * W], fp16, tag="msb", name=f"msb_{j}")
                    for j in range(3)]
            osbs = [opool.tile([128, OSZ], fp16, tag="osb", name=f"osb_{j}")
                    for j in range(3)]
            oscr = [spool.tile([128, TY * W], fp16, tag="oscr", name=f"oscr_{j}")
                    for j in range(4)]
            psts = [ppool.tile([128, NF], mybir.dt.float32, tag="ps", name=f"ps_{j}")
                    for j in range(8)]
            warm_sb = wpool.tile([128, 128], fp16, tag="warm")

            # one-time zeroing: V pad columns (strided 2-col, cheap) and xpad
            # H-pad rows. Only image-0's buffers (0,1) upfront; buffers 2,3
            # are zeroed after image-0's transform is queued (deferred_zero).
            nc.vector.memset(warm_sb[:], 0.0)

            def zero_pads(j, eng=None):
                eng = eng or nc.vector
                vrr = vts[j][:].rearrange("p (t w) -> p t w", w=VROW)
                eng.memset(vrr[:, :, 0:2], 0.0)
                eng.memset(vrr[:, :, 58:60], 0.0)
                # xpad slack tail: written once so AP bounding slices are clean
                eng.memset(xpads[j][:, XIN:XSZ], 0.0)

            zero_pads(0)
            zero_pads(1)

            def load_x(n, ih):
                t = xpads[(n * 2 + ih) % 4]
                eng = nc.scalar if ih == 0 else nc.sync
                eng.dma_start(t[:, 0:XIN], x.ap()[n, ih * 128 : (ih + 1) * 128, :])

            # critical path: image 0 rows 0..29 (covers V chunks 0-1) first
            # on both rings, then weights, then the rest of image 0; image 1
            # is loaded later (from the first chunk-0 block) so its DMA
            # semaphore increments can't inflate image-0 waits
            def load_x0_rows(ih, pr0, pr1):
                # pr = PADDED row indices (0..58); pad rows come from the DMA
                t = xpads[ih % 4]
                eng = nc.scalar if ih == 0 else nc.sync
                eng.dma_start(t[:, pr0 * W : pr1 * W],
                              x.ap()[0, ih * 128 : (ih + 1) * 128, pr0 * W : pr1 * W])

            load_x0_rows(0, 0, 18)
            load_x0_rows(1, 0, 18)
            nc.scalar.dma_start(wts[0][:], wt.ap()[0])
            nc.sync.dma_start(wts[1][:], wt.ap()[1])
            load_x0_rows(0, 18, 31)
            load_x0_rows(1, 18, 31)
            load_x0_rows(0, 31, 58)
            load_x0_rows(1, 31, 58)

            # PE warmup while DMAs land (HAM clock gate 1.2->2.4 GHz)
            warm_ps = psts[7]
            for _ in range(NWARM):
                nc.tensor.matmul(warm_ps[:, 0:128], lhsT=warm_sb[:], rhs=warm_sb[:],
                                 start=True, stop=True)

            def drows(xp, h, ty0, nty):
                # x rows (2*ty + h - 1) for ty in [ty0, ty0+nty): AP [nty, 56]
                off = (2 * ty0 + h) * W
                return xp[:, off : off + nty * 2 * W].rearrange(
                    "p (t w) -> p t w", w=2 * W)[:, :, 0:W]

            def vrows(vt, i, ty0, nty):
                off = (i * TY + ty0) * VROW
                return vt[:, off : off + nty * VROW].rearrange(
                    "p (t w) -> p t w", w=VROW)[:, :, 2 : 2 + W]

            def vop(n, ih, i, ty0, nty):
                xp = xpads[(n * 2 + ih) % 4]
                vt = vts[(n * 2 + ih) % 4]
                d = [drows(xp, h, ty0, nty) for h in range(4)]
                dst = vrows(vt, i, ty0, nty)
                if i == 0:
                    nc.vector.tensor_sub(dst, d[0], d[2])
                elif i == 1:
                    nc.vector.tensor_add(dst, d[1], d[2])
                elif i == 2:
                    nc.vector.tensor_sub(dst, d[2], d[1])
                else:
                    nc.vector.tensor_sub(dst, d[1], d[3])

            def emit_v(n, ih, ty0, nty):
                for i in range(NPL):
                    vop(n, ih, i, ty0, nty)

            def mm_rhs(n, ihf, i, ch, kw):
                vt = vts[(n * 2 + ihf) % 4]
                off = (i * TY + ch * TYC) * VROW
                return vt[:, off : off + TYC * VROW].rearrange(
                    "p (t w) -> p t w", w=VROW)[:, :, kw + 1 : kw + 1 + W]

            # startup: V for image 0 chunk 0 only — the rest is emitted
            # interleaved with chunk matmuls so Tile's engine-semaphore
            # waits stay tight (emitting it all upfront made the first
            # matmuls wait on the whole image-0 transform)
            for i in range(NPL):
                vop(0, 0, i, 0, TYC)
                vop(0, 1, i, 0, TYC)

            def yrows(osb, s, ty0, nty):
                off = ty0 * 2 * W + s * W
                return osb[:, off : off + nty * 2 * W].rearrange(
                    "p (t w) -> p t w", w=2 * W)[:, :, 0:W]

            def emit_combos(n, oh, msb, osb, ty0, nty, sidx):
                # Y[2ty] = m0+m1+m2, Y[2ty+1] = m1-m2-m3 over ty-range
                # (msb chunk-major layout: ty-range == chunk-range * NF)
                m = [msb[:, i * TY * W + ty0 * W : i * TY * W + (ty0 + nty) * W]
                     for i in range(NPL)]
                A = oscr[sidx * 2][:, 0 : nty * W]
                Bs = oscr[sidx * 2 + 1][:, 0 : nty * W]
                nc.vector.tensor_add(A, m[0], m[1])
                nc.vector.tensor_add(yrows(osb, 0, ty0, nty), A, m[2])
                nc.vector.tensor_sub(Bs, m[1], m[2])
                nc.vector.tensor_sub(yrows(osb, 1, ty0, nty), Bs, m[3])

            pp = 0
            for n in range(BPC):
                for oh in range(2):
                    last = (n == BPC - 1 and oh == 1)
                    first = (n == 0 and oh == 0)
                    msb = msbs[(n * 2 + oh) % 3]
                    osb = osbs[(n * 2 + oh) % 3]
                    if not (last or first):
                        for i in range(NPL):
                            # weight-reuse order: same lhsT serves all 4 chunks
                            k = 0
                            for kw in range(3):
                                for ihf in range(2):
                                    c0 = ((i * 3 + kw) * 2 + ihf) * 128
                                    for ch in range(NCH):
                                        nc.tensor.matmul(
                                            psts[(pp + ch) % 8][:],
                                            lhsT=wts[oh][:, c0 : c0 + 128],
                                            rhs=mm_rhs(n, ihf, i, ch, kw),
                                            start=(k == 0), stop=(k == 5))
                                    k += 1
                            for ch in range(NCH):
                                nc.scalar.copy(
                                    msb[:, (i * NCH + ch) * NF :
                                         (i * NCH + ch + 1) * NF],
                                    psts[(pp + ch) % 8][:])
                            pp += NCH
                        emit_combos(n, oh, msb, osb, 0, TY, (n * 2 + oh) % 2)
                        nc.sync.dma_start(
                            out.ap()[n, oh * 128 : (oh + 1) * 128, :],
                            osb[:, 0:HW])
                    else:
                        # first/last (img, oh): chunk-outer so the first
                        # chunk can start before the whole image's V exists
                        # (first) and drains/combos/DMA overlap the matmul
                        # stream, keeping the tail short (last)
                        for ch in range(NCH):
                            tail_ch = last and ch == NCH - 1
                            for i in range(NPL):
                                P = psts[pp % 8]
                                pp += 1
                                k = 0
                                for kw in range(3):
                                    for ihf in range(2):
                                        c0 = ((i * 3 + kw) * 2 + ihf) * 128
                                        nc.tensor.matmul(
                                            P[:], lhsT=wts[oh][:, c0 : c0 + 128],
                                            rhs=mm_rhs(n, ihf, i, ch, kw),
                                            start=(k == 0), stop=(k == 5))
                                        k += 1
                                ms = msb[:, (i * NCH + ch) * NF :
                                         (i * NCH + ch + 1) * NF]
                                if tail_ch and i == NPL - 1:
                                    # last plane drains on DVE: shorter
                                    # PSUM->combo chain after the final MM
                                    nc.vector.tensor_copy(ms, P[:])
                                else:
                                    nc.scalar.copy(ms, P[:])
                            if first:
                                # keep the DVE free for the next chunk's V;
                                # combos for the whole image run later
                                if ch == 0:
                                    zero_pads(2, nc.gpsimd)
                                    zero_pads(3, nc.gpsimd)
                                    load_x(1, 0)
                                    load_x(1, 1)
                                if ch + 1 < NCH:
                                    emit_v(0, 0, (ch + 1) * TYC, TYC)
                                    emit_v(0, 1, (ch + 1) * TYC, TYC)
                                continue
                            ty0 = ch * TYC
                            m = [msb[:, i * TY * W + ty0 * W :
                                     i * TY * W + (ty0 + TYC) * W]
                                 for i in range(NPL)]
                            A = oscr[(ch % 2) * 2][:, 0 : TYC * W]
                            Bs = oscr[(ch % 2) * 2 + 1][:, 0 : TYC * W]
                            nc.vector.tensor_add(A, m[0], m[1])
                            nc.vector.tensor_sub(Bs, m[1], m[2])
                            nc.vector.tensor_add(yrows(osb, 0, ty0, TYC), A, m[2])
                            nc.vector.tensor_sub(yrows(osb, 1, ty0, TYC), Bs, m[3])
                            if tail_ch:
                                nc.sync.dma_start(
                                    out.ap()[n, oh * 128 : (oh + 1) * 128,
                                             ch * TYC * 2 * W : HW],
                                    osb[:, ch * TYC * 2 * W : HW])
                            elif last and ch == 2:
                                nc.sync.dma_start(
                                    out.ap()[n, oh * 128 : (oh + 1) * 128,
                                             2 * TYC * 2 * W : 3 * TYC * 2 * W],
                                    osb[:, 2 * TYC * 2 * W : 3 * TYC * 2 * W])
                            elif ch % 2 == 1:
                                nc.sync.dma_start(
                                    out.ap()[n, oh * 128 : (oh + 1) * 128,
                                             (ch - 1) * TYC * 2 * W :
                                             (ch + 1) * TYC * 2 * W],
                                    osb[:, (ch - 1) * TYC * 2 * W :
                                         (ch + 1) * TYC * 2 * W])
                        if first:
                            emit_combos(n, oh, msb, osb, 0, TY, 0)
                            nc.sync.dma_start(
                                out.ap()[n, oh * 128 : (oh + 1) * 128, :],
                                osb[:, 0:HW])
                    # prefetch next image's V (and x two images ahead)
                    if oh == 0 and n + 1 < BPC:
                        emit_v(n + 1, 0, 0, TY)
                        emit_v(n + 1, 1, 0, TY)
                        if n + 2 < BPC:
                            load_x(n + 2, 0)
                            load_x(n + 2, 1)

    nc.compile()
    return nc


def _pack_weights(weight: np.ndarray) -> np.ndarray:
    bw = np.sign(weight.astype(np.float32))  # [co 256, ci 256, kh 3, kw 3]
    G23 = np.array([[1, 0, 0], [0.5, 0.5, 0.5], [0.5, -0.5, 0.5], [0, 0, 1]],
                   dtype=np.float32)
    U = np.einsum("ik,ockw->iocw", G23, bw)  # [4, co, ci, kw]
    U = U.reshape(NPL, 2, 128, 2, 128, 3)    # [i, ohh, co, ihh, ci, kw]
    U = U.transpose(1, 4, 0, 5, 3, 2)        # [oh, ci, i, kw, ihf, co]
    return np.ascontiguousarray(U.reshape(2, 128, MW)).astype(np.float16)


def _get_nc():
    if "nc" not in _CACHE:
        _CACHE["nc"] = _build_module()
    return _CACHE["nc"]


def _run(x: np.ndarray, weight: np.ndarray, **spmd_kwargs):
    nc = _get_nc()
    wtp = _pack_weights(weight)
    xh = np.zeros((B, C, 58, W), np.float16)
    xh[:, :, 1:57, :] = x.astype(np.float16).reshape(B, C, H, W)
    xh = np.ascontiguousarray(xh.reshape(B, C, 58 * W))
    in_maps = [
        {"x": xh[i * BPC : (i + 1) * BPC], "wt": wtp} for i in range(N_CORES)
    ]
    res = run_bass_kernel_spmd(nc, in_maps, list(range(N_CORES)), **spmd_kwargs)
    out = np.concatenate([r["out"] for r in res.results], axis=0)
    out = out.astype(np.float32).reshape(B, C, H, W)
    return out, res


def kernel(x: np.ndarray, weight: np.ndarray) -> np.ndarray:
    out, _ = _run(x, weight)
    return out


# revision 33
# speedup vs baseline: 1.0334x; 1.0253x over previous
"""Binary-weight 3x3 conv (sign(weight) then conv2d, pad=1) on 8 TRN2 cores.

v2: F(2,3) Winograd along H, direct 3-tap conv along W, fp16 datapath.

Data-parallel over batch: 32 images -> 4 per core; weights replicated.
Per core the conv is decomposed as:
  - input transform along H (DVE): V[i][ci, ty, w] = BT(F2,3) combos of
    x rows 2ty-1..2ty+2 (4 plain tensor_tensor adds per (img, ih-half),
    all +-1 coefficients, fp16 2x perf mode).
  - main matmul (PE): M[i][co, ty, w] = sum_{ci, kw} U[i][co, ci, kw] *
    V[i][ci, ty, w+kw-1], with U = G(F2,3) @ sign(w) along kh (entries in
    {0, +-0.5, +-1, +-1.5}: exact in fp16, packed on host). MAC count is
    2/3 of direct conv: per (img, oh, 7-ty chunk) 24 matmuls of
    [128x128]@[128x392] instead of direct conv's 36.
  - drains (ACT): plain PSUM->SBUF fp16 copies.
  - output transform (DVE): Y[2ty] = m0+m1+m2, Y[2ty+1] = m1-m2-m3
    (4 tensor_tensor ops per (img, oh) over all 28 ty), written
    row-interleaved to fp16 staging; DMA'd out fp16, widened on host.

x is uploaded as fp16 (host cast, like the host-packed weights).
Numerics (numpy bit-accurate sim): rel err 5.4e-4 vs fp32 reference.

Layouts per (img, ih):
  xpad [128, 3304]: H-padded rows r in [-1,56] at offset (r+1)*56 (rows -1
    and 56 zeroed once; 56 trailing slack elems for AP slicing).
  V [128, 4*28*60]: plane i, tile row ty at (i*28+ty)*60; index 2+w holds
    col w in [0,56); indices 1 and 58 are zero W-padding (memset once;
    tap kw reads cols kw+1 .. kw+57 of each row).
"""

import numpy as np

import concourse.bacc as bacc
import concourse.mybir as mybir
import concourse.tile as tile
from concourse.bass_utils import run_bass_kernel_spmd

MODE = "f23h"  # kept for test.py compat

N_CORES = 8
B = 32
BPC = B // N_CORES  # images per core
C = 256
H = W = 56
HW = H * W  # 3136
TY = 28          # H tiles (2 output rows each)
NPL = 4          # transformed planes
VROW = 60        # V row width (2 pad + 56 + 2 pad)
XIN = 58 * W              # 3248: x uploaded host-prepadded (rows -1..56)
XSZ = XIN + W             # 3304: + slack for AP slicing (never read)
VSZ = NPL * TY * VROW     # 6720
OSZ = HW + W              # 3192: osb + slack for strided row writes
TYC = 7          # ty per chunk
NCH = TY // TYC  # 4 chunks per (img, oh)
NF = TYC * W     # 392 matmul free size
MW = NPL * 3 * 2 * 128    # weight cols per oh half
NWARM = 68

_CACHE = {}


def _build_module():
    fp16 = mybir.dt.float16
    nc = bacc.Bacc("TRN2", target_bir_lowering=False, debug=False, num_devices=N_CORES)
    x = nc.declare_dram_parameter("x", [BPC, C, XIN], fp16, isOutput=False)
    wt = nc.declare_dram_parameter("wt", [2, 128, MW], fp16, isOutput=False)
    out = nc.declare_dram_parameter("out", [BPC, C, HW], fp16, isOutput=True)

    with tile.TileContext(nc) as tc:
        with (
            tc.tile_pool(name="xpads", bufs=4) as xpool,
            tc.tile_pool(name="vts", bufs=4) as vpool,
            tc.tile_pool(name="wts", bufs=2) as wpool,
            tc.tile_pool(name="msbs", bufs=3) as mpool,
            tc.tile_pool(name="osbs", bufs=3) as opool,
            tc.tile_pool(name="oscr", bufs=4) as spool,
            tc.tile_pool(name="psum", bufs=8, space="PSUM") as ppool,
        ):
            xpads = [xpool.tile([128, XSZ], fp16, tag="xpad", name=f"xpad_{j}")
                     for j in range(4)]
            vts = [vpool.tile([128, VSZ], fp16, tag="vt", name=f"vt_{j}")
                   for j in range(4)]
            wts = [wpool.tile([128, MW], fp16, tag="wt", name=f"wt_{oh}")
                   for oh in range(2)]
            msbs = [mpool.tile([128, NPL * TY * W], fp16, tag="msb", name=f"msb_{j}")
                    for j in range(3)]
            osbs = [opool.tile([128, OSZ], fp16, tag="osb", name=f"osb_{j}")
                    for j in range(3)]
            oscr = [spool.tile([128, TY * W], fp16, tag="oscr", name=f"oscr_{j}")
                    for j in range(4)]
            psts = [ppool.tile([128, NF], mybir.dt.float32, tag="ps", name=f"ps_{j}")
                    for j in range(8)]
            warm_sb = wpool.tile([128, 128], fp16, tag="warm")

            # one-time zeroing: V pad columns (strided 2-col, cheap) and xpad
            # H-pad rows. Only image-0's buffers (0,1) upfront; buffers 2,3
            # are zeroed after image-0's transform is queued (deferred_zero).
            nc.vector.memset(warm_sb[:], 0.0)

            def zero_pads(j, eng=None):
                eng = eng or nc.vector
                vrr = vts[j][:].rearrange("p (t w) -> p t w", w=VROW)
                eng.memset(vrr[:, :, 0:2], 0.0)
                eng.memset(vrr[:, :, 58:60], 0.0)
                # xpad slack tail: written once so AP bounding slices are clean
                eng.memset(xpads[j][:, XIN:XSZ], 0.0)

            zero_pads(0)
            zero_pads(1)

            def load_x(n, ih):
                t = xpads[(n * 2 + ih) % 4]
                eng = nc.scalar if ih == 0 else nc.sync
                eng.dma_start(t[:, 0:XIN], x.ap()[n, ih * 128 : (ih + 1) * 128, :])

            # critical path: image 0 rows 0..29 (covers V chunks 0-1) first
            # on both rings, then weights, then the rest of image 0; image 1
            # is loaded later (from the first chunk-0 block) so its DMA
            # semaphore increments can't inflate image-0 waits
            def load_x0_rows(ih, pr0, pr1):
                # pr = PADDED row indices (0..58); pad rows come from the DMA
                t = xpads[ih % 4]
                eng = nc.scalar if ih == 0 else nc.sync
                eng.dma_start(t[:, pr0 * W : pr1 * W],
                              x.ap()[0, ih * 128 : (ih + 1) * 128, pr0 * W : pr1 * W])

            load_x0_rows(0, 0, 18)
            load_x0_rows(1, 0, 18)
            # oh0 weights split across both rings so they land before the
            # chunk-0 transform (serial behind one ring they gate the
            # first matmul at ~14.5 us)
            nc.scalar.dma_start(wts[0][:, 0 : MW // 2], wt.ap()[0][:, 0 : MW // 2])
            nc.sync.dma_start(wts[0][:, MW // 2 : MW], wt.ap()[0][:, MW // 2 : MW])
            load_x0_rows(0, 18, 31)
            load_x0_rows(1, 18, 31)
            nc.scalar.dma_start(wts[1][:, 0 : MW // 2], wt.ap()[1][:, 0 : MW // 2])
            nc.sync.dma_start(wts[1][:, MW // 2 : MW], wt.ap()[1][:, MW // 2 : MW])
            load_x0_rows(0, 31, 58)
            load_x0_rows(1, 31, 58)

            # PE warmup while DMAs land (HAM clock gate 1.2->2.4 GHz)
            warm_ps = psts[7]
            for _ in range(NWARM):
                nc.tensor.matmul(warm_ps[:, 0:128], lhsT=warm_sb[:], rhs=warm_sb[:],
                                 start=True, stop=True)

            def drows(xp, h, ty0, nty):
                # x rows (2*ty + h - 1) for ty in [ty0, ty0+nty): AP [nty, 56]
                off = (2 * ty0 + h) * W
                return xp[:, off : off + nty * 2 * W].rearrange(
                    "p (t w) -> p t w", w=2 * W)[:, :, 0:W]

            def vrows(vt, i, ty0, nty):
                off = (i * TY + ty0) * VROW
                return vt[:, off : off + nty * VROW].rearrange(
                    "p (t w) -> p t w", w=VROW)[:, :, 2 : 2 + W]

            def vop(n, ih, i, ty0, nty):
                xp = xpads[(n * 2 + ih) % 4]
                vt = vts[(n * 2 + ih) % 4]
                d = [drows(xp, h, ty0, nty) for h in range(4)]
                dst = vrows(vt, i, ty0, nty)
                if i == 0:
                    nc.vector.tensor_sub(dst, d[0], d[2])
                elif i == 1:
                    nc.vector.tensor_add(dst, d[1], d[2])
                elif i == 2:
                    nc.vector.tensor_sub(dst, d[2], d[1])
                else:
                    nc.vector.tensor_sub(dst, d[1], d[3])

            def emit_v(n, ih, ty0, nty):
                for i in range(NPL):
                    vop(n, ih, i, ty0, nty)

            def mm_rhs(n, ihf, i, ch, kw):
                vt = vts[(n * 2 + ihf) % 4]
                off = (i * TY + ch * TYC) * VROW
                return vt[:, off : off + TYC * VROW].rearrange(
                    "p (t w) -> p t w", w=VROW)[:, :, kw + 1 : kw + 1 + W]

            # startup: V for image 0 chunk 0 only — the rest is emitted
            # interleaved with chunk matmuls so Tile's engine-semaphore
            # waits stay tight (emitting it all upfront made the first
            # matmuls wait on the whole image-0 transform)
            for i in range(NPL):
                vop(0, 0, i, 0, TYC)
                vop(0, 1, i, 0, TYC)

            def yrows(osb, s, ty0, nty):
                off = ty0 * 2 * W + s * W
                return osb[:, off : off + nty * 2 * W].rearrange(
                    "p (t w) -> p t w", w=2 * W)[:, :, 0:W]

            def emit_combos(n, oh, msb, osb, ty0, nty, sidx):
                # Y[2ty] = m0+m1+m2, Y[2ty+1] = m1-m2-m3 over ty-range
                # (msb chunk-major layout: ty-range == chunk-range * NF)
                m = [msb[:, i * TY * W + ty0 * W : i * TY * W + (ty0 + nty) * W]
                     for i in range(NPL)]
                A = oscr[sidx * 2][:, 0 : nty * W]
                Bs = oscr[sidx * 2 + 1][:, 0 : nty * W]
                nc.vector.tensor_add(A, m[0], m[1])
                nc.vector.tensor_add(yrows(osb, 0, ty0, nty), A, m[2])
                nc.vector.tensor_sub(Bs, m[1], m[2])
                nc.vector.tensor_sub(yrows(osb, 1, ty0, nty), Bs, m[3])

            pp = 0
            for n in range(BPC):
                for oh in range(2):
                    last = (n == BPC - 1 and oh == 1)
                    first = (n == 0 and oh == 0)
                    msb = msbs[(n * 2 + oh) % 3]
                    osb = osbs[(n * 2 + oh) % 3]
                    if not (last or first):
                        for i in range(NPL):
                            # weight-reuse order: same lhsT serves all 4 chunks
                            k = 0
                            for kw in range(3):
                                for ihf in range(2):
                                    c0 = ((i * 3 + kw) * 2 + ihf) * 128
                                    for ch in range(NCH):
                                        nc.tensor.matmul(
                                            psts[(pp + ch) % 8][:],
                                            lhsT=wts[oh][:, c0 : c0 + 128],
                                            rhs=mm_rhs(n, ihf, i, ch, kw),
                                            start=(k == 0), stop=(k == 5))
                                    k += 1
                            for ch in range(NCH):
                                nc.scalar.copy(
                                    msb[:, (i * NCH + ch) * NF :
                                         (i * NCH + ch + 1) * NF],
                                    psts[(pp + ch) % 8][:])
                            pp += NCH
                        emit_combos(n, oh, msb, osb, 0, TY, (n * 2 + oh) % 2)
                        nc.sync.dma_start(
                            out.ap()[n, oh * 128 : (oh + 1) * 128, :],
                            osb[:, 0:HW])
                    else:
                        # first/last (img, oh): chunk-outer so the first
                        # chunk can start before the whole image's V exists
                        # (first) and drains/combos/DMA overlap the matmul
                        # stream, keeping the tail short (last)
                        for ch in range(NCH):
                            tail_ch = last and ch == NCH - 1
                            for i in range(NPL):
                                P = psts[pp % 8]
                                pp += 1
                                k = 0
                                for kw in range(3):
                                    for ihf in range(2):
                                        c0 = ((i * 3 + kw) * 2 + ihf) * 128
                                        nc.tensor.matmul(
                                            P[:], lhsT=wts[oh][:, c0 : c0 + 128],
                                            rhs=mm_rhs(n, ihf, i, ch, kw),
                                            start=(k == 0), stop=(k == 5))
                                        k += 1
                                ms = msb[:, (i * NCH + ch) * NF :
                                         (i * NCH + ch + 1) * NF]
                                if tail_ch and i == NPL - 1:
                                    # last plane drains on DVE: shorter
                                    # PSUM->combo chain after the final MM
                                    nc.vector.tensor_copy(ms, P[:])
                                else:
                                    nc.scalar.copy(ms, P[:])
                            if first:
                                # keep the DVE free for the next chunk's V;
                                # combos for the whole image run later
                                if ch == 0:
                                    zero_pads(2, nc.gpsimd)
                                    zero_pads(3, nc.gpsimd)
                                    load_x(1, 0)
                                    load_x(1, 1)
                                if ch + 1 < NCH:
                                    emit_v(0, 0, (ch + 1) * TYC, TYC)
                                    emit_v(0, 1, (ch + 1) * TYC, TYC)
                                continue
                            ty0 = ch * TYC
                            m = [msb[:, i * TY * W + ty0 * W :
                                     i * TY * W + (ty0 + TYC) * W]
                                 for i in range(NPL)]
                            A = oscr[(ch % 2) * 2][:, 0 : TYC * W]
                            Bs = oscr[(ch % 2) * 2 + 1][:, 0 : TYC * W]
                            nc.vector.tensor_add(A, m[0], m[1])
                            nc.vector.tensor_sub(Bs, m[1], m[2])
                            nc.vector.tensor_add(yrows(osb, 0, ty0, TYC), A, m[2])
                            nc.vector.tensor_sub(yrows(osb, 1, ty0, TYC), Bs, m[3])
                            if tail_ch:
                                nc.sync.dma_start(
                                    out.ap()[n, oh * 128 : (oh + 1) * 128,
                                             ch * TYC * 2 * W : HW],
                                    osb[:, ch * TYC * 2 * W : HW])
                            elif last and ch == 2:
                                nc.sync.dma_start(
                                    out.ap()[n, oh * 128 : (oh + 1) * 128,
                                             2 * TYC * 2 * W : 3 * TYC * 2 * W],
                                    osb[:, 2 * TYC * 2 * W : 3 * TYC * 2 * W])
                            elif ch % 2 == 1:
                                nc.sync.dma_start(
                                    out.ap()[n, oh * 128 : (oh + 1) * 128,
                                             (ch - 1) * TYC * 2 * W :
                                             (ch + 1) * TYC * 2 * W],
                                    osb[:, (ch - 1) * TYC * 2 * W :
                                         (ch + 1) * TYC * 2 * W])
                        if first:
                            emit_combos(n, oh, msb, osb, 0, TY, 0)
                            nc.sync.dma_start(
                                out.ap()[n, oh * 128 : (oh + 1) * 128, :],
                                osb[:, 0:HW])
                    # prefetch next image's V (and x two images ahead)
                    if oh == 0 and n + 1 < BPC:
                        emit_v(n + 1, 0, 0, TY)
                        emit_v(n + 1, 1, 0, TY)
                        if n + 2 < BPC:
                            load_x(n + 2, 0)
                            load_x(n + 2, 1)

    nc.compile()
    return nc


def _pack_weights(weight: np.ndarray) -> np.ndarray:
    bw = np.sign(weight.astype(np.float32))  # [co 256, ci 256, kh 3, kw 3]
    G23 = np.array([[1, 0, 0], [0.5, 0.5, 0.5], [0.5, -0.5, 0.5], [0, 0, 1]],
                   dtype=np.float32)
    U = np.einsum("ik,ockw->iocw", G23, bw)  # [4, co, ci, kw]
    U = U.reshape(NPL, 2, 128, 2, 128, 3)    # [i, ohh, co, ihh, ci, kw]
    U = U.transpose(1, 4, 0, 5, 3, 2)        # [oh, ci, i, kw, ihf, co]
    return np.ascontiguousarray(U.reshape(2, 128, MW)).astype(np.float16)


def _get_nc():
    if "nc" not in _CACHE:
        _CACHE["nc"] = _build_module()
    return _CACHE["nc"]


def _run(x: np.ndarray, weight: np.ndarray, **spmd_kwargs):
    nc = _get_nc()
    wtp = _pack_weights(weight)
    xh = np.zeros((B, C, 58, W), np.float16)
    xh[:, :, 1:57, :] = x.astype(np.float16).reshape(B, C, H, W)
    xh = np.ascontiguousarray(xh.reshape(B, C, 58 * W))
    in_maps = [
        {"x": xh[i * BPC : (i + 1) * BPC], "wt": wtp} for i in range(N_CORES)
    ]
    res = run_bass_kernel_spmd(nc, in_maps, list(range(N_CORES)), **spmd_kwargs)
    out = np.concatenate([r["out"] for r in res.results], axis=0)
    out = out.astype(np.float32).reshape(B, C, H, W)
    return out, res


def kernel(x: np.ndarray, weight: np.ndarray) -> np.ndarray:
    out, _ = _run(x, weight)
    return out
